# revision 9
# baseline (speedup 1.0000x reference)
"""BinaryConv (binary-weight 3x3 conv) on 8 Trainium2 NeuronCores.

Full-input contract: kernel(x=[32,256,56,56] f32, weight=[256,256,3,3] f32)
-> [32,256,56,56] f32.

Strategy: data-parallel over batch (4 images/core), weight replicated.
Per core, a 1D Winograd F(4,3) decomposition ALONG H (direct taps along W):
for each H-tile of 4 output rows, 6 Winograd components l replace the 9-tap
sum with 6 comps x 3 W-taps = 18 matmul-rows per 4 output rows vs 36 direct
-- half the PE work.  Per (l, kw): out_wino[l] += Wwino[l,kw]^T . uH[l]
where uH[l] = B^T-combination of 6 input rows (computed on DVE in bf16) and
Wwino[l,kw] = G-combination of sign(w) taps (exact ints scaled once).  The
fp32 scale a[o]=mean|w[o]| applies at PSUM eviction; the inverse transform
y = A^T m runs on Pool/DVE in bf16; output ships bf16 and is upcast to f32
on the host (lossless marshalling).

Host-side marshalling (layout/dtype only, all math on device): x ships bf16,
weight ships both as f32 [O,I,3,3] (feeds the |w| reduction on Pool) and as
a tap-major bf16 transpose [9,I,O] whose sign feeds the Wwino combination
(sign(bf16(w)) == sign(w)).
"""

import ml_dtypes
import numpy as np

import concourse.mybir as mybir
import concourse.tile as tile
from concourse import bacc
from concourse.bass_utils import run_bass_kernel_spmd

F32 = mybir.dt.float32
BF16 = mybir.dt.bfloat16
ALU = mybir.AluOpType

N_CORES = 8
B, C, H, W = 32, 256, 56, 56
O, KH, KW = 256, 3, 3
BP = B // N_CORES            # images per core
P = 128                      # partitions
NCI = C // P                 # input-channel chunks
NCO = O // P                 # output-channel chunks
NL = 6                       # Winograd F(4,3) components along H
M = 4                        # output rows per H-tile
IT = H // M                  # 14 H-tiles
IB = IT // 2                 # 7 H-tiles per psum block (28 output rows)
NFREE = IB * W               # 392 <= 512 fp32 psum bank
KIN = C * KH * KW            # 2304 per-filter fan-in
PH = H + 2                   # padded rows -1..56


def build(bp: int = BP):
    nc = bacc.Bacc(
        "TRN2",
        target_bir_lowering=False,
        debug=False,
        enable_asserts=False,
        num_devices=N_CORES,
        enable_partition_id=False,
    )
    x_d = nc.dram_tensor("x", [bp, C, H, W], BF16, kind="ExternalInput")
    w_d = nc.dram_tensor("w", [O, C, KH, KW], F32, kind="ExternalInput")
    # wp[t, i, o] = bf16(w[o, i, t]) -- host-marshalled tap-major transpose
    wp_d = nc.dram_tensor("wp", [KH * KW, C, O], BF16, kind="ExternalInput")
    out_d = nc.dram_tensor("out", [bp, O, H, W], BF16, kind="ExternalOutput")

    x = x_d.ap().rearrange("n c h w -> n c (h w)")
    w = w_d.ap().rearrange("o i kh kw -> o (i kh kw)")
    wp = wp_d.ap().rearrange("t (c p) o -> p c t o", p=P)
    out = out_d.ap()

    with tile.TileContext(nc) as tc:
        with (
            tc.tile_pool(name="const", bufs=1) as cpool,
            tc.tile_pool(name="wstage", bufs=2) as wspool,
            tc.tile_pool(name="wtmp", bufs=4) as wtpool,
            tc.tile_pool(name="xt", bufs=4) as xpool,
            tc.tile_pool(name="uh", bufs=4) as upool,
            tc.tile_pool(name="ft", bufs=2) as fpool,
            tc.tile_pool(name="mev", bufs=12) as mpool,
            tc.tile_pool(name="itmp", bufs=2) as ipool,
            tc.tile_pool(name="yt", bufs=4) as ypool,
            tc.tile_pool(name="psum", bufs=7, space="PSUM") as pspool,
            tc.tile_pool(name="warmps", bufs=1, space="PSUM") as wppool,
        ):
            # ---- PE warmup: hold HAM clock while inputs stream in --------
            warm_l = cpool.tile([P, P], BF16)
            warm_r = cpool.tile([P, 512], BF16)
            nc.gpsimd.memset(warm_l[:], 0.0)
            nc.gpsimd.memset(warm_r[:], 0.0)
            zbias = cpool.tile([P, 1], F32)
            zscr = cpool.tile([P, 1], F32)
            nc.gpsimd.memset(zbias[:], 0.0)
            warm_ps = wppool.tile([P, 512], F32)
            for _ in range(14):
                nc.tensor.matmul(warm_ps[:], warm_l[:], warm_r[:],
                                 start=True, stop=True)
            # preload the Sign LUT on ACT before the weights arrive
            nc.scalar.sign(zscr[:], zbias[:], bias=zbias[:])

            # ---- critical-path input DMAs on the sync ring (FIFO) --------
            wps = cpool.tile([P, NCI, KH, KW, O], BF16, name="wps")
            for c1 in range(NCI):
                nc.sync.dma_start(
                    wps[:, c1].rearrange("p kh kw o -> p (kh kw) o"),
                    wp[:, c1])

            def x_alloc():
                xt = xpool.tile([P, PH, W], BF16, name="xt")
                nc.gpsimd.memset(xt[:, 0, :], 0.0)
                nc.gpsimd.memset(xt[:, PH - 1, :], 0.0)
                return xt

            def x_load(n):
                ts = []
                for c1 in range(NCI):
                    xt = x_alloc()
                    nc.sync.dma_start(
                        xt[:, 1:PH - 1, :].rearrange("p h w -> p (h w)"),
                        x[n, c1 * P:(c1 + 1) * P, :])
                    ts.append(xt)
                return ts

            xts = x_load(0)

            wstages = [wspool.tile([P, KIN], F32, name="ws")
                       for _ in range(NCO)]
            nc.sync.dma_start(wstages[0][:], w[0:P, :])
            nc.sync.dma_start(wstages[1][:], w[P:2 * P, :])

            # ---- |w| means via ACT abs+accumulate (off the DVE path) -----
            a_all = cpool.tile([P, NCO], F32)
            wscr = wspool.tile([P, KIN], BF16, name="wscr")
            for co in range(NCO):
                asum = wspool.tile([P, 1], F32, name="asum", bufs=2)
                nc.scalar.activation(
                    wscr[:], wstages[co][:],
                    mybir.ActivationFunctionType.Abs,
                    bias=zbias[:], accum_out=asum[:])
                nc.gpsimd.tensor_scalar_mul(
                    a_all[:, co:co + 1], asum[:], 1.0 / KIN)

            # ---- sign (ACT), kh-chunked ----------------------------------
            wsg = cpool.tile([P, NCI, KH, KW, O], BF16, name="wsg")
            for kh in range(KH):
                nc.scalar.sign(wsg[:, :, kh], wps[:, :, kh], bias=zbias[:])

            # ---- Wwino combos on DVE: wt[l] = G-combination of sign taps -
            # G rows: [1/4,0,0], -(s0+s1+s2)/6, (s1-s0-s2)/6,
            #         (s0+2s1+4s2)/24, (s0-2s1+4s2)/24, [0,0,1]
            # l=5 is s2 exactly -> matmuls read wsg[:, :, 2, kw] directly.
            wt = cpool.tile([P, NCI, NL - 1, KW, O], BF16, name="wt")

            def s_(kh, kw):
                return wsg[:, :, kh, kw]      # [P, NCI, O]

            for kw in range(KW):
                nc.vector.tensor_scalar_mul(
                    wt[:, :, 0, kw], s_(0, kw), 0.25)
            for kw in range(KW):
                q = wtpool.tile([P, NCI, O], BF16, name="wq")
                nc.vector.tensor_tensor(
                    q[:], s_(0, kw), s_(1, kw), op=ALU.add)
                nc.vector.tensor_tensor(
                    q[:], q[:], s_(2, kw), op=ALU.add)
                nc.vector.tensor_scalar_mul(wt[:, :, 1, kw], q[:], -1.0 / 6)
            for kw in range(KW):
                q = wtpool.tile([P, NCI, O], BF16, name="wq")
                nc.vector.tensor_tensor(
                    q[:], s_(1, kw), s_(0, kw), op=ALU.subtract)
                nc.vector.tensor_tensor(
                    q[:], q[:], s_(2, kw), op=ALU.subtract)
                nc.vector.tensor_scalar_mul(wt[:, :, 2, kw], q[:], 1.0 / 6)
            for kw in range(KW):
                q = wtpool.tile([P, NCI, O], BF16, name="wq")
                r = wtpool.tile([P, NCI, O], BF16, name="wr")
                nc.vector.scalar_tensor_tensor(
                    q[:], s_(1, kw), 2.0, s_(0, kw),
                    op0=ALU.mult, op1=ALU.add)
                nc.vector.scalar_tensor_tensor(
                    r[:], s_(2, kw), 4.0, q[:], op0=ALU.mult, op1=ALU.add)
                nc.vector.tensor_scalar_mul(wt[:, :, 3, kw], r[:], 1.0 / 24)
                nc.vector.scalar_tensor_tensor(
                    q[:], s_(1, kw), -2.0, s_(0, kw),
                    op0=ALU.mult, op1=ALU.add)
                nc.vector.scalar_tensor_tensor(
                    r[:], s_(2, kw), 4.0, q[:], op0=ALU.mult, op1=ALU.add)
                nc.vector.tensor_scalar_mul(wt[:, :, 4, kw], r[:], 1.0 / 24)

            def lhsT(c1, l, kw, co):
                if l == NL - 1:
                    return wsg[:, c1, 2, kw, co * P:(co + 1) * P]
                return wt[:, c1, l, kw, co * P:(co + 1) * P]

            # ---- forward transform: uH[l] = B^T . rows, on DVE -----------
            def fwd(xts_n, c1, l_major):
                xt = xts_n[c1]
                uh = upool.tile([P, NL, IT, W + 2], BF16, name="uh")
                nc.gpsimd.memset(uh[:, :, :, 0], 0.0)
                nc.gpsimd.memset(uh[:, :, :, W + 1], 0.0)
                xv = [xt[:, r:r + 53:4, :] for r in range(6)]
                ft = fpool.tile([P, 8, IT, W], BF16, name="ft")

                def u(l):
                    return uh[:, l, :, 1:W + 1]

                def ops(l):
                    if l == 0:
                        nc.vector.scalar_tensor_tensor(
                            ft[:, 6], xv[2], -5.0, xv[4],
                            op0=ALU.mult, op1=ALU.add)
                        nc.vector.scalar_tensor_tensor(
                            u(0), xv[0], 4.0, ft[:, 6],
                            op0=ALU.mult, op1=ALU.add)
                    elif l == 1:
                        nc.vector.tensor_tensor(
                            ft[:, 0], xv[1], xv[2], op=ALU.add)
                        nc.vector.tensor_tensor(
                            ft[:, 1], xv[3], xv[4], op=ALU.add)
                        nc.vector.scalar_tensor_tensor(
                            u(1), ft[:, 0], -4.0, ft[:, 1],
                            op0=ALU.mult, op1=ALU.add)
                    elif l == 2:
                        nc.vector.tensor_tensor(
                            ft[:, 2], xv[1], xv[2], op=ALU.subtract)
                        nc.vector.tensor_tensor(
                            ft[:, 3], xv[4], xv[3], op=ALU.subtract)
                        nc.vector.scalar_tensor_tensor(
                            u(2), ft[:, 2], 4.0, ft[:, 3],
                            op0=ALU.mult, op1=ALU.add)
                    elif l == 3:
                        nc.vector.tensor_tensor(
                            ft[:, 4], xv[1], xv[3], op=ALU.subtract)
                        nc.vector.tensor_tensor(
                            ft[:, 5], xv[4], xv[2], op=ALU.subtract)
                        nc.vector.scalar_tensor_tensor(
                            u(3), ft[:, 4], -2.0, ft[:, 5],
                            op0=ALU.mult, op1=ALU.add)
                    elif l == 4:
                        nc.vector.scalar_tensor_tensor(
                            u(4), ft[:, 4], 2.0, ft[:, 5],
                            op0=ALU.mult, op1=ALU.add)
                    else:
                        nc.vector.scalar_tensor_tensor(
                            ft[:, 7], xv[3], -5.0, xv[5],
                            op0=ALU.mult, op1=ALU.add)
                        nc.vector.scalar_tensor_tensor(
                            u(5), xv[1], 4.0, ft[:, 7],
                            op0=ALU.mult, op1=ALU.add)

                if not l_major:
                    for l in range(NL):
                        ops(l)
                return uh, ops

            # n=0: interleave the two chunks l-major so the PE can start
            # on the l=0 psum group before the rest of uH lands.
            uhs = [None, None]
            pend = []
            for c1 in range(NCI):
                uhs[c1], op_emit = fwd(xts, c1, l_major=True)
                pend.append(op_emit)
            for l in range(NL):
                for c1 in range(NCI):
                    pend[c1](l)

            # ---- main loop ----------------------------------------------
            for n in range(bp):
                if n + 1 < bp:
                    nxt_xts = x_load(n + 1)
                nxt_uhs = [None, None]
                for co in range(NCO):
                    for hb in range(2):
                        i0 = hb * IB
                        ms = []
                        for l in range(NL):
                            ps = pspool.tile([P, IB, W], F32, name="ps")
                            for c1 in range(NCI):
                                for kw in range(KW):
                                    nc.tensor.matmul(
                                        ps[:],
                                        lhsT(c1, l, kw, co),
                                        uhs[c1][:, l, i0:i0 + IB, kw:kw + W],
                                        start=(c1 == 0 and kw == 0),
                                        stop=(c1 == NCI - 1 and kw == KW - 1),
                                    )
                            m = mpool.tile([P, IB, W], BF16, name="m")
                            sc = a_all[:, co:co + 1]
                            if l in (1, 3):
                                nc.vector.tensor_scalar_mul(m[:], ps[:], sc)
                            else:
                                nc.scalar.mul(m[:], ps[:], sc)
                            ms.append(m)
                        # inverse transform y = A^T m (Pool temps, DVE tail)
                        e = ipool.tile([P, IB, W], BF16, name="e")
                        o_ = ipool.tile([P, IB, W], BF16, name="o")
                        f = ipool.tile([P, IB, W], BF16, name="f")
                        g = ipool.tile([P, IB, W], BF16, name="g")
                        nc.gpsimd.tensor_tensor(
                            e[:], ms[1][:], ms[2][:], op=ALU.add)
                        nc.gpsimd.tensor_tensor(
                            o_[:], ms[1][:], ms[2][:], op=ALU.subtract)
                        nc.gpsimd.tensor_tensor(
                            f[:], ms[3][:], ms[4][:], op=ALU.add)
                        nc.gpsimd.tensor_tensor(
                            g[:], ms[3][:], ms[4][:], op=ALU.subtract)
                        y = ypool.tile([P, IB * M, W], BF16, name="y")
                        t = ipool.tile([P, IB, W], BF16, name="t")
                        t2 = ipool.tile([P, IB, W], BF16, name="t2")
                        nc.gpsimd.tensor_tensor(
                            t[:], ms[0][:], e[:], op=ALU.add)

                        def yv(r):
                            return y[:, r::M, :]

                        nc.vector.tensor_tensor(
                            yv(0), t[:], f[:], op=ALU.add)
                        nc.vector.scalar_tensor_tensor(
                            yv(1), g[:], 2.0, o_[:],
                            op0=ALU.mult, op1=ALU.add)
                        nc.vector.scalar_tensor_tensor(
                            yv(2), f[:], 4.0, e[:],
                            op0=ALU.mult, op1=ALU.add)
                        nc.gpsimd.tensor_tensor(
                            t2[:], ms[5][:], o_[:], op=ALU.add)
                        nc.vector.scalar_tensor_tensor(
                            yv(3), g[:], 8.0, t2[:],
                            op0=ALU.mult, op1=ALU.add)
                        nc.scalar.dma_start(
                            out[n, co * P:(co + 1) * P,
                                i0 * M:(i0 + IB) * M, :],
                            y[:])
                    # overlap next image's forward transform with this one
                    if n + 1 < bp:
                        c1 = co
                        nxt_uhs[c1], _ = fwd(nxt_xts, c1, l_major=False)
                if n + 1 < bp:
                    uhs = nxt_uhs

    nc.compile()
    return nc


_NC_CACHE: dict[int, object] = {}


def _get_nc(bp: int = BP):
    if bp not in _NC_CACHE:
        _NC_CACHE[bp] = build(bp)
    return _NC_CACHE[bp]


def make_in_maps(x: np.ndarray, weight: np.ndarray, n_cores: int = N_CORES,
                 bp: int = BP):
    x = np.ascontiguousarray(x, dtype=np.float32)
    weight = np.ascontiguousarray(weight, dtype=np.float32)
    xb = x.astype(ml_dtypes.bfloat16)
    wp = np.ascontiguousarray(
        weight.reshape(O, C, KH * KW).transpose(2, 1, 0)
    ).astype(ml_dtypes.bfloat16)  # [t, i, o]
    return [
        {"x": xb[i * bp:(i + 1) * bp], "w": weight, "wp": wp}
        for i in range(n_cores)
    ]


def kernel(x: np.ndarray, weight: np.ndarray) -> np.ndarray:
    nc = _get_nc(BP)
    in_maps = make_in_maps(x, weight)
    res = run_bass_kernel_spmd(nc, in_maps, core_ids=list(range(N_CORES)))
    out = np.empty((B, O, H, W), dtype=np.float32)
    for i in range(N_CORES):
        out[i * BP:(i + 1) * BP] = (
            res.results[i]["out"].astype(np.float32).reshape(BP, O, H, W))
    return out


# revision 12
# speedup vs baseline: 1.1568x; 1.1568x over previous
"""BinaryConv (binary-weight 3x3 conv) on 8 Trainium2 NeuronCores.

Full-input contract: kernel(x=[32,256,56,56] f32, weight=[256,256,3,3] f32)
-> [32,256,56,56] f32.

Strategy: data-parallel over batch (4 images/core), weight replicated.
Per core, a 1D Winograd F(4,3) decomposition ALONG H (direct taps along W):
for each H-tile of 4 output rows, 6 Winograd components l replace the 9-tap
sum with 6 comps x 3 W-taps = 18 matmul-rows per 4 output rows vs 36 direct
-- half the PE work.  Per (l, kw): out_wino[l] += Wwino[l,kw]^T . uH[l].

uH[l] = B^T . input rows, computed on DVE via three shifted full-row helper
tensors (A1=x[t]+x[t+1] odd rows, D1=x[t]-x[t+1] odd rows, D2=x[t]-x[t+2])
followed by one fused scalar_tensor_tensor per component:
  u0 = 4*D2[4i]   - D2[4i+2]      u1 = -4*A1[4i+1] + A1[4i+3]
  u2 = 4*D1[4i+1] - D1[4i+3]      u3 = -2*D2[4i+1] - D2[4i+2]
  u4 = 2*D2[4i+1] - D2[4i+2]      u5 = 4*D2[4i+1]  - D2[4i+3]
All DVE APs are <=3D, innermost step 1, 4B-aligned (2x_1p perf mode).

Wwino[l,kw] = G-combination of sign(w) taps (exact ints, one bf16 scale).
PSUM accumulates l-pairs in [P,2,512] tiles (2 banks); ACT evicts each pair
in one op fused with the fp32 scale a[o]=mean|w[o]|.  The inverse
y = A^T m runs per (n,co) on Pool (temps) + DVE (outputs); output ships
bf16 and is upcast to f32 on the host (lossless marshalling).

Host-side marshalling (layout/dtype only, all math on device): x ships bf16;
weight ships as a tap-major bf16 transpose [9,I,O] (sign source;
sign(bf16(w)) == sign(w)) and as bf16 [O, I*9] feeding the |w| mean (the
bf16 rounding of |w| averages out over the 2304-element mean).
"""

import ml_dtypes
import numpy as np

import concourse.mybir as mybir
import concourse.tile as tile
from concourse import bacc
from concourse.bass_utils import run_bass_kernel_spmd

F32 = mybir.dt.float32
BF16 = mybir.dt.bfloat16
ALU = mybir.AluOpType
ACTF = mybir.ActivationFunctionType

N_CORES = 8
B, C, H, W = 32, 256, 56, 56
O, KH, KW = 256, 3, 3
BP = B // N_CORES            # images per core
P = 128                      # partitions
NCI = C // P                 # input-channel chunks
NCO = O // P                 # output-channel chunks
NL = 6                       # Winograd F(4,3) components along H
M = 4                        # output rows per H-tile
IT = H // M                  # 14 H-tiles
IB = IT // 2                 # 7 H-tiles per psum half-block
NFREE = IB * W               # 392 <= 512 fp32 psum bank
KIN = C * KH * KW            # 2304 per-filter fan-in
PH = H + 2                   # padded rows -1..56
UW = W + 4                   # uh row: [skip, pad, 56 interior, pad]


def build(bp: int = BP):
    nc = bacc.Bacc(
        "TRN2",
        target_bir_lowering=False,
        debug=False,
        enable_asserts=False,
        num_devices=N_CORES,
        enable_partition_id=False,
    )
    x_d = nc.dram_tensor("x", [bp, C, H, W], BF16, kind="ExternalInput")
    # wp[t, i, o] = bf16(w[o, i, t]); wb[o, fan] = bf16(w[o]) flat
    wp_d = nc.dram_tensor("wp", [KH * KW, C, O], BF16, kind="ExternalInput")
    wb_d = nc.dram_tensor("wb", [O, KIN], BF16, kind="ExternalInput")
    out_d = nc.dram_tensor("out", [bp, O, H, W], BF16, kind="ExternalOutput")

    x = x_d.ap().rearrange("n c h w -> n c (h w)")
    wp = wp_d.ap().rearrange("t (c p) o -> p c t o", p=P)
    wb = wb_d.ap()
    out = out_d.ap()

    with tile.TileContext(nc) as tc:
        with (
            tc.tile_pool(name="const", bufs=1) as cpool,
            tc.tile_pool(name="wtmp", bufs=1) as wtpool,
            tc.tile_pool(name="xt", bufs=2) as xpool,
            tc.tile_pool(name="uh", bufs=2) as upool,
            tc.tile_pool(name="ft", bufs=1) as fpool,
            tc.tile_pool(name="mev", bufs=2) as mpool,
            tc.tile_pool(name="itmp", bufs=1) as ipool,
            tc.tile_pool(name="yt", bufs=2) as ypool,
            tc.tile_pool(name="psum", bufs=4, space="PSUM") as pspool,
        ):
            # ---- PE warmup: hold HAM clock while inputs stream in --------
            warm_l = cpool.tile([P, P], BF16)
            warm_r = cpool.tile([P, 512], BF16)
            nc.gpsimd.memset(warm_l[:], 0.0)
            nc.gpsimd.memset(warm_r[:], 0.0)
            zbias = cpool.tile([P, 1], F32)
            zscr = cpool.tile([P, 1], F32)
            nc.gpsimd.memset(zbias[:], 0.0)
            warm_ps = pspool.tile([P, 2, 512], F32, name="ps")
            for _ in range(14):
                nc.tensor.matmul(warm_ps[:, 0], warm_l[:], warm_r[:],
                                 start=True, stop=True)
            # preload the Sign LUT on ACT before the weights arrive
            nc.scalar.sign(zscr[:], zbias[:], bias=zbias[:])

            # ---- critical-path input DMAs on the sync ring (FIFO) --------
            wsg = cpool.tile([P, NCI, KH, KW, O], BF16, name="wsg")
            for c1 in range(NCI):
                nc.sync.dma_start(
                    wsg[:, c1].rearrange("p kh kw o -> p (kh kw) o"),
                    wp[:, c1])

            def x_load(n):
                xt = xpool.tile([P, NCI, PH, W], BF16, name="xt")
                nc.gpsimd.memset(xt[:, :, 0, :], 0.0)
                nc.gpsimd.memset(xt[:, :, PH - 1, :], 0.0)
                for c1 in range(NCI):
                    nc.sync.dma_start(
                        xt[:, c1, 1:PH - 1, :].rearrange("p h w -> p (h w)"),
                        x[n, c1 * P:(c1 + 1) * P, :])
                return xt

            xt0 = x_load(0)

            wstages = [cpool.tile([P, KIN], BF16, name=f"ws{co}")
                       for co in range(NCO)]
            nc.sync.dma_start(wstages[0][:], wb[0:P, :])
            nc.sync.dma_start(wstages[1][:], wb[P:2 * P, :])

            # ---- sign in place (ACT), kh-chunked -------------------------
            for kh in range(KH):
                nc.scalar.sign(wsg[:, :, kh], wsg[:, :, kh], bias=zbias[:])

            # ---- |w| means via ACT abs+accumulate (in place) -------------
            a_all = cpool.tile([P, NCO], F32)
            asums = [cpool.tile([P, 1], F32, name=f"as{co}")
                     for co in range(NCO)]
            for co in range(NCO):
                nc.scalar.activation(
                    wstages[co][:], wstages[co][:], ACTF.Abs,
                    bias=zbias[:], accum_out=asums[co][:])

            # ---- Wwino combos on DVE: wt[l] = G-combination of sign taps -
            # G rows: s0/4, -(s0+s1+s2)/6, (s1-s0-s2)/6,
            #         (s0+2s1+4s2)/24, (s0-2s1+4s2)/24, s2
            # l=5 is s2 exactly -> matmuls read wsg[:, :, 2, kw] directly.
            wt = cpool.tile([P, NCI, NL - 1, KW, O], BF16, name="wt")

            def s_(kh, kw):
                return wsg[:, :, kh, kw]      # [P, NCI, O]

            def combos(l):
                if l == 0:
                    for kw in range(KW):
                        nc.vector.tensor_scalar_mul(
                            wt[:, :, 0, kw], s_(0, kw), 0.25)
                    # a scales (tiny, needed by the first eviction)
                    for co in range(NCO):
                        nc.vector.tensor_scalar_mul(
                            a_all[:, co:co + 1], asums[co][:], 1.0 / KIN)
                elif l == 1:
                    for kw in range(KW):
                        q = wtpool.tile([P, NCI, O], BF16, name="wq")
                        nc.vector.tensor_tensor(
                            q[:], s_(0, kw), s_(1, kw), op=ALU.add)
                        nc.vector.tensor_tensor(
                            q[:], q[:], s_(2, kw), op=ALU.add)
                        nc.vector.tensor_scalar_mul(
                            wt[:, :, 1, kw], q[:], -1.0 / 6)
                elif l == 2:
                    for kw in range(KW):
                        q = wtpool.tile([P, NCI, O], BF16, name="wq2")
                        nc.vector.tensor_tensor(
                            q[:], s_(1, kw), s_(0, kw), op=ALU.subtract)
                        nc.vector.tensor_tensor(
                            q[:], q[:], s_(2, kw), op=ALU.subtract)
                        nc.vector.tensor_scalar_mul(
                            wt[:, :, 2, kw], q[:], 1.0 / 6)
                elif l in (3, 4):
                    sg = 2.0 if l == 3 else -2.0
                    for kw in range(KW):
                        q = wtpool.tile([P, NCI, O], BF16, name=f"wq{l}")
                        nc.vector.scalar_tensor_tensor(
                            q[:], s_(1, kw), sg, s_(0, kw),
                            op0=ALU.mult, op1=ALU.add)
                        nc.vector.scalar_tensor_tensor(
                            q[:], s_(2, kw), 4.0, q[:],
                            op0=ALU.mult, op1=ALU.add)
                        nc.vector.tensor_scalar_mul(
                            wt[:, :, l, kw], q[:], 1.0 / 24)

            def lhsT(c1, l, kw, co):
                if l == NL - 1:
                    return wsg[:, c1, 2, kw, co * P:(co + 1) * P]
                return wt[:, c1, l, kw, co * P:(co + 1) * P]

            # ---- forward transform on DVE (see module docstring) ---------
            def fwd(xt, interleave=None):
                """interleave: optional callback(l) run after each comp's
                ops are emitted (used to thread weight combos at startup)."""
                uh = upool.tile([P, NCI, NL, IT, UW], BF16, name="uh")
                nc.gpsimd.memset(uh[:, :, :, :, 1], 0.0)
                nc.gpsimd.memset(uh[:, :, :, :, W + 2], 0.0)
                a1 = fpool.tile([P, NCI, 28, W], BF16, name="a1")
                d1 = fpool.tile([P, NCI, 28, W], BF16, name="d1")
                d2 = fpool.tile([P, NCI, H, W], BF16, name="d2")

                def u(l, c1):
                    return uh[:, c1, l, :, 2:W + 2]

                def stt(o, i0, s, op1, i1):
                    nc.vector.scalar_tensor_tensor(
                        o, i0, s, i1, op0=ALU.mult, op1=op1)

                for c1 in range(NCI):
                    xc = xt[:, c1]
                    nc.vector.tensor_tensor(
                        d2[:, c1].rearrange("p h w -> p (h w)"),
                        xc[:, 0:H, :].rearrange("p h w -> p (h w)"),
                        xc[:, 2:PH, :].rearrange("p h w -> p (h w)"),
                        op=ALU.subtract)
                    stt(u(0, c1), d2[:, c1, 0:53:4, :], 4.0,
                        ALU.subtract, d2[:, c1, 2:55:4, :])
                    if interleave and c1 == NCI - 1:
                        interleave(0)
                for c1 in range(NCI):
                    xc = xt[:, c1]
                    nc.vector.tensor_tensor(
                        a1[:, c1], xc[:, 1:56:2, :], xc[:, 2:57:2, :],
                        op=ALU.add)
                    stt(u(1, c1), a1[:, c1, 0:27:2, :], -4.0,
                        ALU.add, a1[:, c1, 1:28:2, :])
                    if interleave and c1 == NCI - 1:
                        interleave(1)
                for c1 in range(NCI):
                    xc = xt[:, c1]
                    nc.vector.tensor_tensor(
                        d1[:, c1], xc[:, 1:56:2, :], xc[:, 2:57:2, :],
                        op=ALU.subtract)
                    stt(u(2, c1), d1[:, c1, 0:27:2, :], 4.0,
                        ALU.subtract, d1[:, c1, 1:28:2, :])
                    if interleave and c1 == NCI - 1:
                        interleave(2)
                for c1 in range(NCI):
                    stt(u(3, c1), d2[:, c1, 1:54:4, :], -2.0,
                        ALU.subtract, d2[:, c1, 2:55:4, :])
                    if interleave and c1 == NCI - 1:
                        interleave(3)
                for c1 in range(NCI):
                    stt(u(4, c1), d2[:, c1, 1:54:4, :], 2.0,
                        ALU.subtract, d2[:, c1, 2:55:4, :])
                    if interleave and c1 == NCI - 1:
                        interleave(4)
                for c1 in range(NCI):
                    stt(u(5, c1), d2[:, c1, 1:54:4, :], 4.0,
                        ALU.subtract, d2[:, c1, 3:56:4, :])
                return uh

            uhs = fwd(xt0, interleave=combos)

            # ---- main loop ----------------------------------------------
            for n in range(bp):
                if n + 1 < bp:
                    nxt_xt = x_load(n + 1)
                m = mpool.tile([P, NL, NCO, 2, NFREE], BF16, name="m")
                for co in range(NCO):
                    for hb in range(2):
                        i0 = hb * IB
                        for lp in range(NL // 2):
                            ps = pspool.tile([P, 2, 512], F32, name="ps")
                            for j in range(2):
                                l = 2 * lp + j
                                for c1 in range(NCI):
                                    for kw in range(KW):
                                        nc.tensor.matmul(
                                            ps[:, j, :NFREE],
                                            lhsT(c1, l, kw, co),
                                            uhs[:, c1, l, i0:i0 + IB,
                                                kw + 1:kw + 1 + W],
                                            start=(c1 == 0 and kw == 0),
                                            stop=(c1 == NCI - 1
                                                  and kw == KW - 1),
                                        )
                            # fused evict+scale of the l-pair on ACT
                            nc.scalar.mul(
                                m[:, 2 * lp:2 * lp + 2, co, hb, :],
                                ps[:, :, :NFREE],
                                a_all[:, co:co + 1])
                    # inverse transform y = A^T m for this co (Pool+DVE)
                    mv = [m[:, l, co] for l in range(NL)]  # [P, 2, NFREE]
                    e = ipool.tile([P, 2, NFREE], BF16, name="e")
                    o_ = ipool.tile([P, 2, NFREE], BF16, name="o")
                    f = ipool.tile([P, 2, NFREE], BF16, name="f")
                    g = ipool.tile([P, 2, NFREE], BF16, name="g")
                    t = ipool.tile([P, 2, NFREE], BF16, name="t")
                    nc.gpsimd.tensor_tensor(e[:], mv[1], mv[2], op=ALU.add)
                    nc.gpsimd.tensor_tensor(
                        o_[:], mv[1], mv[2], op=ALU.subtract)
                    nc.gpsimd.tensor_tensor(f[:], mv[3], mv[4], op=ALU.add)
                    nc.gpsimd.tensor_tensor(
                        g[:], mv[3], mv[4], op=ALU.subtract)
                    nc.gpsimd.tensor_tensor(t[:], mv[0], e[:], op=ALU.add)
                    if co == 0:
                        y = ypool.tile([P, NCO, H, W], BF16, name="y")

                    def yv(r):
                        return y[:, co, r::M, :]     # [P, 14, 56]

                    nc.vector.tensor_tensor(yv(0), t[:], f[:], op=ALU.add)
                    nc.vector.scalar_tensor_tensor(
                        yv(1), g[:], 2.0, o_[:], op0=ALU.mult, op1=ALU.add)
                    nc.vector.scalar_tensor_tensor(
                        yv(2), f[:], 4.0, e[:], op0=ALU.mult, op1=ALU.add)
                    nc.vector.tensor_tensor(t[:], mv[5], o_[:], op=ALU.add)
                    nc.vector.scalar_tensor_tensor(
                        yv(3), g[:], 8.0, t[:], op0=ALU.mult, op1=ALU.add)
                    nc.scalar.dma_start(
                        out[n, co * P:(co + 1) * P, :, :], y[:, co])
                    # overlap next image's forward transform
                    if n + 1 < bp and co == 0:
                        nxt_uh = fwd(nxt_xt)
                if n + 1 < bp:
                    uhs = nxt_uh

    nc.compile()
    return nc


_NC_CACHE: dict[int, object] = {}


def _get_nc(bp: int = BP):
    if bp not in _NC_CACHE:
        _NC_CACHE[bp] = build(bp)
    return _NC_CACHE[bp]


def make_in_maps(x: np.ndarray, weight: np.ndarray, n_cores: int = N_CORES,
                 bp: int = BP):
    x = np.ascontiguousarray(x, dtype=np.float32)
    weight = np.ascontiguousarray(weight, dtype=np.float32)
    xb = x.astype(ml_dtypes.bfloat16)
    wp = np.ascontiguousarray(
        weight.reshape(O, C, KH * KW).transpose(2, 1, 0)
    ).astype(ml_dtypes.bfloat16)  # [t, i, o]
    wb = weight.reshape(O, KIN).astype(ml_dtypes.bfloat16)
    return [
        {"x": xb[i * bp:(i + 1) * bp], "wp": wp, "wb": wb}
        for i in range(n_cores)
    ]


def kernel(x: np.ndarray, weight: np.ndarray) -> np.ndarray:
    nc = _get_nc(BP)
    in_maps = make_in_maps(x, weight)
    res = run_bass_kernel_spmd(nc, in_maps, core_ids=list(range(N_CORES)))
    out = np.empty((B, O, H, W), dtype=np.float32)
    for i in range(N_CORES):
        out[i * BP:(i + 1) * BP] = (
            res.results[i]["out"].astype(np.float32).reshape(BP, O, H, W))
    return out


# revision 14
# speedup vs baseline: 1.2449x; 1.0762x over previous
"""BinaryConv (binary-weight 3x3 conv) on 8 Trainium2 NeuronCores.

Full-input contract: kernel(x=[32,256,56,56] f32, weight=[256,256,3,3] f32)
-> [32,256,56,56] f32.

Strategy: data-parallel over batch (4 images/core), weight replicated.
Per core, a 1D Winograd F(4,3) decomposition ALONG H (direct taps along W):
for each H-tile of 4 output rows, 6 Winograd components l replace the 9-tap
sum with 6 comps x 3 W-taps = 18 matmul-rows per 4 output rows vs 36 direct
-- half the PE work.  Per (l, kw): out_wino[l] += Wwino[l,kw]^T . uH[l].

x ships in PHASE-MAJOR row order (host layout only): plane p holds padded
rows t=4i+p, pre-padded in H and W, so every forward-transform op is a
fully-contiguous 2D DVE op (2x_1p perf mode).  With Y_p = plane p rows
[0:14], Z_p = rows [1:15]:
  tc = Z0-Y2   w1 = Y0-Y2  tp = Y1-Y3  q5 = Y3-Z1
  q1 = Y1+Y2   q2 = Y3+Z0  r1 = Y1-Y2  r2 = Y3-Z0
  u0 = 4*w1+tc  u1 = -4*q1+q2  u2 = 4*r1-r2
  u3 = -2*tp+tc u4 = 2*tp+tc   u5 = 4*tp-q5
Pad columns come in as zeros inside each plane row, so uH needs no memsets.

Wwino[l,kw] = G-combination of sign(w) taps (exact ints, one bf16 scale).
PSUM accumulates l-pairs in [P,2,512] tiles (2 banks); ACT evicts each pair
in one op fused with the fp32 scale a[o]=mean|w[o]|.  The inverse
y = A^T m runs co-merged on Pool (e,o) + DVE (rest), writing a phase-major
y that the host de-phases; output ships bf16 and is upcast to f32 on the
host (lossless marshalling).

Host-side marshalling (layout/dtype/zero-pad only, all math on device):
x ships bf16 phase-major; weight ships as a tap-major bf16 transpose
[9,I,O] (sign source; sign(bf16(w)) == sign(w)) and as bf16 [O, I*9]
feeding the |w| mean (bf16 rounding of |w| averages out over the mean).
"""

import ml_dtypes
import numpy as np

import concourse.mybir as mybir
import concourse.tile as tile
from concourse import bacc
from concourse.bass_utils import run_bass_kernel_spmd

F32 = mybir.dt.float32
BF16 = mybir.dt.bfloat16
ALU = mybir.AluOpType
ACTF = mybir.ActivationFunctionType

N_CORES = 8
B, C, H, W = 32, 256, 56, 56
O, KH, KW = 256, 3, 3
BP = B // N_CORES            # images per core
P = 128                      # partitions
NCI = C // P                 # input-channel chunks
NCO = O // P                 # output-channel chunks
NL = 6                       # Winograd F(4,3) components along H
M = 4                        # output rows per H-tile
IT = H // M                  # 14 H-tiles
IB = IT // 2                 # 7 H-tiles per psum half-block
NFREE = IB * W               # 392 <= 512 fp32 psum bank
KIN = C * KH * KW            # 2304 per-filter fan-in
W2 = W + 2                   # padded row width
NPI = IT + 1                 # rows per phase plane
PSZ = NPI * NCI * W2         # phase plane size = 1740
USZ = IT * NCI * W2          # uh component size = 1624


def build(bp: int = BP):
    nc = bacc.Bacc(
        "TRN2",
        target_bir_lowering=False,
        debug=False,
        enable_asserts=False,
        num_devices=N_CORES,
        enable_partition_id=False,
    )
    # x4[n, pc, p, i, c1, w] = xpad[n, c1*128+pc, 4i+p, w] (rows -1..58+)
    x_d = nc.dram_tensor("x4", [bp, P, M, NPI, NCI, W2], BF16,
                         kind="ExternalInput")
    wp_d = nc.dram_tensor("wp", [KH * KW, C, O], BF16, kind="ExternalInput")
    wb_d = nc.dram_tensor("wb", [O, KIN], BF16, kind="ExternalInput")
    # out4[n, o, r, i, w] = out[n, o, 4i+r, w]
    out_d = nc.dram_tensor("out", [bp, O, M, IT, W], BF16,
                           kind="ExternalOutput")

    x = x_d.ap().rearrange("n k p i c w -> n k (p i c w)")
    wp = wp_d.ap().rearrange("t (c p) o -> p c t o", p=P)
    wb = wb_d.ap()
    out = out_d.ap()

    with tile.TileContext(nc) as tc:
        with (
            tc.tile_pool(name="const", bufs=1) as cpool,
            tc.tile_pool(name="wtmp", bufs=1) as wtpool,
            tc.tile_pool(name="xt", bufs=2) as xpool,
            tc.tile_pool(name="uh", bufs=2) as upool,
            tc.tile_pool(name="ft", bufs=1) as fpool,
            tc.tile_pool(name="mev", bufs=2) as mpool,
            tc.tile_pool(name="itmp", bufs=1) as ipool,
            tc.tile_pool(name="yt", bufs=2) as ypool,
            tc.tile_pool(name="psum", bufs=4, space="PSUM") as pspool,
        ):
            # ---- PE warmup: hold HAM clock through the startup ramp ------
            warm_l = cpool.tile([P, P], BF16)
            warm_r = cpool.tile([P, 512], BF16)
            nc.gpsimd.memset(warm_l[:], 0.0)
            nc.gpsimd.memset(warm_r[:], 0.0)
            zbias = cpool.tile([P, 1], F32)
            zscr = cpool.tile([P, 1], F32)
            nc.gpsimd.memset(zbias[:], 0.0)
            warm_ps = pspool.tile([P, 2, 512], F32, name="ps")
            for _ in range(14):
                nc.tensor.matmul(warm_ps[:, 0], warm_l[:], warm_r[:],
                                 start=True, stop=True)
            for _ in range(110):
                nc.tensor.matmul(warm_ps[:, 0, :128], warm_l[:],
                                 warm_r[:, :128], start=True, stop=True)
            # preload the Sign LUT on ACT before the weights arrive
            nc.scalar.sign(zscr[:], zbias[:], bias=zbias[:])

            # ---- critical-path input DMAs on the sync ring (FIFO) --------
            wsg = cpool.tile([P, NCI, KH, KW, O], BF16, name="wsg")
            for c1 in range(NCI):
                nc.sync.dma_start(
                    wsg[:, c1].rearrange("p kh kw o -> p (kh kw) o"),
                    wp[:, c1])

            def x_load(n):
                xt = xpool.tile([P, M, NPI, NCI, W2], BF16, name="xt")
                nc.sync.dma_start(
                    xt[:].rearrange("p m i c w -> p (m i c w)"), x[n])
                return xt

            xt0 = x_load(0)

            wstages = [cpool.tile([P, KIN], BF16, name=f"ws{co}")
                       for co in range(NCO)]
            nc.sync.dma_start(wstages[0][:], wb[0:P, :])
            nc.sync.dma_start(wstages[1][:], wb[P:2 * P, :])

            # ---- sign in place (ACT), kh-chunked -------------------------
            for kh in range(KH):
                nc.scalar.sign(wsg[:, :, kh], wsg[:, :, kh], bias=zbias[:])

            # ---- |w| means via ACT abs+accumulate (in place) -------------
            a_all = cpool.tile([P, NCO], F32)
            asums = [cpool.tile([P, 1], F32, name=f"as{co}")
                     for co in range(NCO)]
            for co in range(NCO):
                nc.scalar.activation(
                    wstages[co][:], wstages[co][:], ACTF.Abs,
                    bias=zbias[:], accum_out=asums[co][:])

            # ---- Wwino combos on DVE: wt[l] = G-combination of sign taps -
            # G rows: s0/4, -(s0+s1+s2)/6, (s1-s0-s2)/6,
            #         (s0+2s1+4s2)/24, (s0-2s1+4s2)/24, s2
            # l=5 is s2 exactly -> matmuls read wsg[:, :, 2, kw] directly.
            wt = cpool.tile([P, NCI, NL - 1, KW, O], BF16, name="wt")

            def s_(kh, kw):
                return wsg[:, :, kh, kw]      # [P, NCI, O]

            def combos(l):
                if l == 0:
                    for kw in range(KW):
                        nc.vector.tensor_scalar_mul(
                            wt[:, :, 0, kw], s_(0, kw), 0.25)
                    for co in range(NCO):
                        nc.vector.tensor_scalar_mul(
                            a_all[:, co:co + 1], asums[co][:], 1.0 / KIN)
                elif l == 1:
                    for kw in range(KW):
                        q = wtpool.tile([P, NCI, O], BF16, name="wq")
                        nc.vector.tensor_tensor(
                            q[:], s_(0, kw), s_(1, kw), op=ALU.add)
                        nc.vector.tensor_tensor(
                            q[:], q[:], s_(2, kw), op=ALU.add)
                        nc.vector.tensor_scalar_mul(
                            wt[:, :, 1, kw], q[:], -1.0 / 6)
                elif l == 2:
                    for kw in range(KW):
                        q = wtpool.tile([P, NCI, O], BF16, name="wq2")
                        nc.vector.tensor_tensor(
                            q[:], s_(1, kw), s_(0, kw), op=ALU.subtract)
                        nc.vector.tensor_tensor(
                            q[:], q[:], s_(2, kw), op=ALU.subtract)
                        nc.vector.tensor_scalar_mul(
                            wt[:, :, 2, kw], q[:], 1.0 / 6)
                elif l in (3, 4):
                    sg = 2.0 if l == 3 else -2.0
                    for kw in range(KW):
                        q = wtpool.tile([P, NCI, O], BF16, name=f"wq{l}")
                        nc.vector.scalar_tensor_tensor(
                            q[:], s_(1, kw), sg, s_(0, kw),
                            op0=ALU.mult, op1=ALU.add)
                        nc.vector.scalar_tensor_tensor(
                            q[:], s_(2, kw), 4.0, q[:],
                            op0=ALU.mult, op1=ALU.add)
                        nc.vector.tensor_scalar_mul(
                            wt[:, :, l, kw], q[:], 1.0 / 24)

            def lhsT(c1, l, kw, co):
                if l == NL - 1:
                    return wsg[:, c1, 2, kw, co * P:(co + 1) * P]
                return wt[:, c1, l, kw, co * P:(co + 1) * P]

            # ---- forward transform (see module docstring) ----------------
            def fwd(xt, startup=False, interleave=None):
                uh = upool.tile([P, NL, IT, NCI, W2], BF16, name="uh")
                ft = [fpool.tile([P, USZ], BF16, name=f"f{k}")
                      for k in range(5)]
                fA, fB, fC, fD, fE = ft

                def pl(p, s):     # plane p rows [s : s+14], flat [P, USZ]
                    return xt[:, p, s:s + IT].rearrange(
                        "p i c w -> p (i c w)")

                def u(l):
                    return uh[:, l].rearrange("p i c w -> p (i c w)")

                def stt(o, i0, s, op1, i1):
                    nc.vector.scalar_tensor_tensor(
                        o, i0, s, i1, op0=ALU.mult, op1=op1)

                Y0, Y1, Y2, Y3 = pl(0, 0), pl(1, 0), pl(2, 0), pl(3, 0)
                Z0, Z1 = pl(0, 1), pl(1, 1)
                tt = nc.vector.tensor_tensor
                il = interleave if interleave else (lambda l: None)
                if not startup:
                    # q1/r1 on Pool, issued first (feed u1/u2 later)
                    nc.gpsimd.tensor_tensor(fD[:], Y1, Y2, op=ALU.add)
                    nc.gpsimd.tensor_tensor(fE[:], Y1, Y2, op=ALU.subtract)
                il(0)
                tt(fA[:], Z0, Y2, op=ALU.subtract)      # tc
                tt(fB[:], Y0, Y2, op=ALU.subtract)      # w1
                stt(u(0), fB[:], 4.0, ALU.add, fA[:])
                il(1)
                if startup:
                    tt(fD[:], Y1, Y2, op=ALU.add)       # q1
                tt(fB[:], Y3, Z0, op=ALU.add)           # q2
                stt(u(1), fD[:], -4.0, ALU.add, fB[:])
                il(2)
                if startup:
                    tt(fE[:], Y1, Y2, op=ALU.subtract)  # r1
                tt(fB[:], Y3, Z0, op=ALU.subtract)      # r2
                stt(u(2), fE[:], 4.0, ALU.subtract, fB[:])
                il(3)
                tt(fC[:], Y1, Y3, op=ALU.subtract)      # tp
                stt(u(3), fC[:], -2.0, ALU.add, fA[:])
                il(4)
                stt(u(4), fC[:], 2.0, ALU.add, fA[:])
                tt(fB[:], Y3, Z1, op=ALU.subtract)      # q5
                stt(u(5), fC[:], 4.0, ALU.subtract, fB[:])
                return uh

            uhs = fwd(xt0, startup=True, interleave=combos)

            # ---- main loop ----------------------------------------------
            for n in range(bp):
                last = n + 1 == bp
                if not last:
                    nxt_xt = x_load(n + 1)
                m = mpool.tile([P, NL, NCO, 2, NFREE], BF16, name="m")
                for co in range(NCO):
                    for hb in range(2):
                        i0 = hb * IB
                        for lp in range(NL // 2):
                            ps = pspool.tile([P, 2, 512], F32, name="ps")
                            for j in range(2):
                                l = 2 * lp + j
                                for c1 in range(NCI):
                                    for kw in range(KW):
                                        nc.tensor.matmul(
                                            ps[:, j, :NFREE],
                                            lhsT(c1, l, kw, co),
                                            uhs[:, l, i0:i0 + IB, c1,
                                                kw:kw + W],
                                            start=(c1 == 0 and kw == 0),
                                            stop=(c1 == NCI - 1
                                                  and kw == KW - 1),
                                        )
                            # fused evict+scale of the l-pair on ACT
                            nc.scalar.mul(
                                m[:, 2 * lp:2 * lp + 2, co, hb, :],
                                ps[:, :, :NFREE],
                                a_all[:, co:co + 1])
                    if not last and co == 0:
                        nxt_uh = fwd(nxt_xt)
                # inverse transform y = A^T m (co-merged; per-co on last
                # image to shorten the tail), Pool computes e/o.
                cosl = [slice(None)] if not last else \
                    [slice(co, co + 1) for co in range(NCO)]
                y = ypool.tile([P, M, NCO, IT, W], BF16, name="y")
                for cs in cosl:
                    ncs = NCO if cs == slice(None) else 1
                    sz = ncs * 2 * NFREE
                    mv = [m[:, l, cs].rearrange("p c h w -> p (c h w)")
                          for l in range(NL)]
                    e = ipool.tile([P, NCO, 2, NFREE], BF16, name="e")
                    o_ = ipool.tile([P, NCO, 2, NFREE], BF16, name="o")
                    f = ipool.tile([P, NCO, 2, NFREE], BF16, name="f")
                    g = ipool.tile([P, NCO, 2, NFREE], BF16, name="g")
                    t = ipool.tile([P, NCO, 2, NFREE], BF16, name="t")
                    ev = e[:].rearrange("p c h w -> p (c h w)")[:, :sz]
                    ov = o_[:].rearrange("p c h w -> p (c h w)")[:, :sz]
                    fv = f[:].rearrange("p c h w -> p (c h w)")[:, :sz]
                    gv = g[:].rearrange("p c h w -> p (c h w)")[:, :sz]
                    tv = t[:].rearrange("p c h w -> p (c h w)")[:, :sz]
                    itt = nc.vector.tensor_tensor if last else \
                        nc.gpsimd.tensor_tensor
                    itt(ev, mv[1], mv[2], op=ALU.add)
                    itt(ov, mv[1], mv[2], op=ALU.subtract)
                    nc.vector.tensor_tensor(fv, mv[3], mv[4], op=ALU.add)
                    nc.vector.tensor_tensor(
                        gv, mv[3], mv[4], op=ALU.subtract)
                    nc.vector.tensor_tensor(tv, mv[0], ev, op=ALU.add)

                    def yv(r):
                        return y[:, r, cs].rearrange("p c h w -> p (c h w)")

                    nc.vector.tensor_tensor(yv(0), tv, fv, op=ALU.add)
                    nc.vector.scalar_tensor_tensor(
                        yv(1), gv, 2.0, ov, op0=ALU.mult, op1=ALU.add)
                    nc.vector.scalar_tensor_tensor(
                        yv(2), fv, 4.0, ev, op0=ALU.mult, op1=ALU.add)
                    nc.vector.tensor_tensor(tv, mv[5], ov, op=ALU.add)
                    nc.vector.scalar_tensor_tensor(
                        yv(3), gv, 8.0, tv, op0=ALU.mult, op1=ALU.add)
                for co in range(NCO):
                    nc.scalar.dma_start(
                        out[n, co * P:(co + 1) * P, :, :, :], y[:, :, co])
                if not last:
                    uhs = nxt_uh

    nc.compile()
    return nc


_NC_CACHE: dict[int, object] = {}


def _get_nc(bp: int = BP):
    if bp not in _NC_CACHE:
        _NC_CACHE[bp] = build(bp)
    return _NC_CACHE[bp]


def make_in_maps(x: np.ndarray, weight: np.ndarray, n_cores: int = N_CORES,
                 bp: int = BP):
    x = np.ascontiguousarray(x, dtype=np.float32)
    weight = np.ascontiguousarray(weight, dtype=np.float32)
    xb = x.astype(ml_dtypes.bfloat16)
    # phase-major padded layout: x4[n, pc, p, i, c1, w2]
    xpad = np.zeros((B, C, M * NPI, W2), dtype=ml_dtypes.bfloat16)
    xpad[:, :, 1:H + 1, 1:W + 1] = xb
    x4 = np.ascontiguousarray(
        xpad.reshape(B, NCI, P, NPI, M, W2).transpose(0, 2, 4, 3, 1, 5))
    wp = np.ascontiguousarray(
        weight.reshape(O, C, KH * KW).transpose(2, 1, 0)
    ).astype(ml_dtypes.bfloat16)  # [t, i, o]
    wb = weight.reshape(O, KIN).astype(ml_dtypes.bfloat16)
    return [
        {"x4": x4[i * bp:(i + 1) * bp], "wp": wp, "wb": wb}
        for i in range(n_cores)
    ]


def kernel(x: np.ndarray, weight: np.ndarray) -> np.ndarray:
    nc = _get_nc(BP)
    in_maps = make_in_maps(x, weight)
    res = run_bass_kernel_spmd(nc, in_maps, core_ids=list(range(N_CORES)))
    out = np.empty((B, O, H, W), dtype=np.float32)
    for i in range(N_CORES):
        o4 = res.results[i]["out"].astype(np.float32)      # [bp,O,4,14,56]
        out[i * BP:(i + 1) * BP] = (
            o4.reshape(BP, O, M, IT, W).transpose(0, 1, 3, 2, 4)
            .reshape(BP, O, H, W))
    return out


# revision 15
# speedup vs baseline: 1.4038x; 1.1277x over previous
"""BinaryConv (binary-weight 3x3 conv) on 8 Trainium2 NeuronCores.

Full-input contract: kernel(x=[32,256,56,56] f32, weight=[256,256,3,3] f32)
-> [32,256,56,56] f32.

Strategy: data-parallel over batch (4 images/core), weight replicated.
Per core, a 1D Winograd F(4,3) decomposition ALONG H (direct taps along W):
for each H-tile of 4 output rows, 6 Winograd components l replace the 9-tap
sum with 6 comps x 3 W-taps = 18 matmul-rows per 4 output rows vs 36 direct
-- half the PE work.  Per (l, kw): out_wino[l] += Wwino[l,kw]^T . uH[l].

x ships in PHASE-MAJOR row order (host layout only): plane p holds padded
rows t=4i+p, pre-padded in H and W, so every forward-transform op is a
fully-contiguous 2D DVE op (2x_1p perf mode).  With Y_p = plane p rows
[0:14], Z_p = rows [1:15]:
  tc = Z0-Y2   w1 = Y0-Y2  tp = Y1-Y3  q5 = Y3-Z1
  q1 = Y1+Y2   q2 = Y3+Z0  r1 = Y1-Y2  r2 = Y3-Z0
  u0 = 4*w1+tc  u1 = -4*q1+q2  u2 = 4*r1-r2
  u3 = -2*tp+tc u4 = 2*tp+tc   u5 = 4*tp-q5
Pad columns come in as zeros inside each plane row, so uH needs no memsets.

Wwino[l,kw] = G-combination of sign(w) taps (exact ints, one bf16 scale).
PSUM accumulates l-pairs in [P,2,512] tiles (2 banks); ACT evicts each pair
in one op fused with the fp32 scale a[o]=mean|w[o]|.  The inverse
y = A^T m runs co-merged on Pool (e,o) + DVE (rest), writing a phase-major
y that the host de-phases; output ships bf16 and is upcast to f32 on the
host (lossless marshalling).

Host-side marshalling (layout/dtype/zero-pad only, all math on device):
x ships bf16 phase-major; weight ships as a tap-major bf16 transpose
[9,I,O] (sign source; sign(bf16(w)) == sign(w)) and as bf16 [O, I*9]
feeding the |w| mean (bf16 rounding of |w| averages out over the mean).
"""

import ml_dtypes
import numpy as np

import concourse.mybir as mybir
import concourse.tile as tile
from concourse import bacc
from concourse.bass_utils import run_bass_kernel_spmd

F32 = mybir.dt.float32
BF16 = mybir.dt.bfloat16
ALU = mybir.AluOpType
ACTF = mybir.ActivationFunctionType

N_CORES = 8
B, C, H, W = 32, 256, 56, 56
O, KH, KW = 256, 3, 3
BP = B // N_CORES            # images per core
P = 128                      # partitions
NCI = C // P                 # input-channel chunks
NCO = O // P                 # output-channel chunks
NL = 6                       # Winograd F(4,3) components along H
M = 4                        # output rows per H-tile
IT = H // M                  # 14 H-tiles
IB = IT // 2                 # 7 H-tiles per psum half-block
NFREE = IB * W               # 392 <= 512 fp32 psum bank
KIN = C * KH * KW            # 2304 per-filter fan-in
W2 = W + 2                   # padded row width
NPI = IT + 1                 # rows per phase plane
PSZ = NPI * NCI * W2         # phase plane size = 1740
USZ = IT * NCI * W2          # uh component size = 1624


def build(bp: int = BP):
    nc = bacc.Bacc(
        "TRN2",
        target_bir_lowering=False,
        debug=False,
        enable_asserts=False,
        num_devices=N_CORES,
        enable_partition_id=False,
    )
    # x4[n, pc, p, i, c1, w] = xpad[n, c1*128+pc, 4i+p, w] (rows -1..58+)
    x_d = nc.dram_tensor("x4", [bp, P, M, NPI, NCI, W2], BF16,
                         kind="ExternalInput")
    wp_d = nc.dram_tensor("wp", [KH * KW, C, O], BF16, kind="ExternalInput")
    wb_d = nc.dram_tensor("wb", [O, KIN], BF16, kind="ExternalInput")
    # out4[n, o, r, i, w] = out[n, o, 4i+r, w]
    out_d = nc.dram_tensor("out", [bp, O, M, IT, W], BF16,
                           kind="ExternalOutput")

    x = x_d.ap().rearrange("n k p i c w -> n k (p i c w)")
    wp = wp_d.ap().rearrange("t (c p) o -> p c t o", p=P)
    wb = wb_d.ap()
    out = out_d.ap()

    with tile.TileContext(nc) as tc:
        with (
            tc.tile_pool(name="const", bufs=1) as cpool,
            tc.tile_pool(name="wtmp", bufs=1) as wtpool,
            tc.tile_pool(name="xt", bufs=2) as xpool,
            tc.tile_pool(name="uh", bufs=2) as upool,
            tc.tile_pool(name="ft", bufs=1) as fpool,
            tc.tile_pool(name="mev", bufs=2) as mpool,
            tc.tile_pool(name="itmp", bufs=1) as ipool,
            tc.tile_pool(name="yt", bufs=2) as ypool,
            tc.tile_pool(name="psum", bufs=4, space="PSUM") as pspool,
        ):
            # ---- PE warmup: hold HAM clock through the startup ramp ------
            warm_l = cpool.tile([P, P], BF16)
            warm_r = cpool.tile([P, 512], BF16)
            nc.gpsimd.memset(warm_l[:], 0.0)
            nc.gpsimd.memset(warm_r[:], 0.0)
            zbias = cpool.tile([P, 1], F32)
            zscr = cpool.tile([P, 1], F32)
            nc.gpsimd.memset(zbias[:], 0.0)
            warm_ps = pspool.tile([P, 2, 512], F32, name="ps")
            for _ in range(14):
                nc.tensor.matmul(warm_ps[:, 0], warm_l[:], warm_r[:],
                                 start=True, stop=True)
            for _ in range(110):
                nc.tensor.matmul(warm_ps[:, 0, :128], warm_l[:],
                                 warm_r[:, :128], start=True, stop=True)
            # preload the Sign LUT on ACT before the weights arrive
            nc.scalar.sign(zscr[:], zbias[:], bias=zbias[:])

            # ---- critical-path input DMAs on the sync ring (FIFO) --------
            wsg = cpool.tile([P, NCI, KH, KW, O], BF16, name="wsg")
            for c1 in range(NCI):
                nc.sync.dma_start(
                    wsg[:, c1].rearrange("p kh kw o -> p (kh kw) o"),
                    wp[:, c1])

            def x_load(n):
                xt = xpool.tile([P, M, NPI, NCI, W2], BF16, name="xt")
                nc.sync.dma_start(
                    xt[:].rearrange("p m i c w -> p (m i c w)"), x[n])
                return xt

            xt0 = x_load(0)

            wstages = [cpool.tile([P, KIN], BF16, name=f"ws{co}")
                       for co in range(NCO)]
            nc.sync.dma_start(wstages[0][:], wb[0:P, :])
            nc.sync.dma_start(wstages[1][:], wb[P:2 * P, :])

            # ---- sign in place (ACT), kh-chunked -------------------------
            for kh in range(KH):
                nc.scalar.sign(wsg[:, :, kh], wsg[:, :, kh], bias=zbias[:])

            # ---- |w| means via ACT abs+accumulate (in place) -------------
            a_all = cpool.tile([P, NCO], F32)
            asums = [cpool.tile([P, 1], F32, name=f"as{co}")
                     for co in range(NCO)]
            for co in range(NCO):
                nc.scalar.activation(
                    wstages[co][:], wstages[co][:], ACTF.Abs,
                    bias=zbias[:], accum_out=asums[co][:])

            # ---- Wwino combos on DVE: wt[l] = G-combination of sign taps -
            # G rows: s0/4, -(s0+s1+s2)/6, (s1-s0-s2)/6,
            #         (s0+2s1+4s2)/24, (s0-2s1+4s2)/24, s2
            # l=5 is s2 exactly -> matmuls read wsg[:, :, 2, kw] directly.
            wt = cpool.tile([P, NCI, NL - 1, KW, O], BF16, name="wt")

            def s_(kh, kw):
                return wsg[:, :, kh, kw]      # [P, NCI, O]

            def combos(l):
                if l == 0:
                    for kw in range(KW):
                        nc.vector.tensor_scalar_mul(
                            wt[:, :, 0, kw], s_(0, kw), 0.25)
                    for co in range(NCO):
                        nc.vector.tensor_scalar_mul(
                            a_all[:, co:co + 1], asums[co][:], 1.0 / KIN)
                elif l == 1:
                    for kw in range(KW):
                        q = wtpool.tile([P, NCI, O], BF16, name="wq")
                        nc.vector.tensor_tensor(
                            q[:], s_(0, kw), s_(1, kw), op=ALU.add)
                        nc.vector.tensor_tensor(
                            q[:], q[:], s_(2, kw), op=ALU.add)
                        nc.vector.tensor_scalar_mul(
                            wt[:, :, 1, kw], q[:], -1.0 / 6)
                elif l == 2:
                    for kw in range(KW):
                        q = wtpool.tile([P, NCI, O], BF16, name="wq2")
                        nc.vector.tensor_tensor(
                            q[:], s_(1, kw), s_(0, kw), op=ALU.subtract)
                        nc.vector.tensor_tensor(
                            q[:], q[:], s_(2, kw), op=ALU.subtract)
                        nc.vector.tensor_scalar_mul(
                            wt[:, :, 2, kw], q[:], 1.0 / 6)
                elif l in (3, 4):
                    sg = 2.0 if l == 3 else -2.0
                    for kw in range(KW):
                        q = wtpool.tile([P, NCI, O], BF16, name=f"wq{l}")
                        nc.vector.scalar_tensor_tensor(
                            q[:], s_(1, kw), sg, s_(0, kw),
                            op0=ALU.mult, op1=ALU.add)
                        nc.vector.scalar_tensor_tensor(
                            q[:], s_(2, kw), 4.0, q[:],
                            op0=ALU.mult, op1=ALU.add)
                        nc.vector.tensor_scalar_mul(
                            wt[:, :, l, kw], q[:], 1.0 / 24)

            def lhsT(c1, l, kw, co):
                if l == NL - 1:
                    return wsg[:, c1, 2, kw, co * P:(co + 1) * P]
                return wt[:, c1, l, kw, co * P:(co + 1) * P]

            # ---- forward transform (see module docstring) ----------------
            # STT has no 2x uop; use prescale (DVE TS @4x / ACT mul) + TT.
            def fwd(xt, startup=False, interleave=None):
                uh = upool.tile([P, NL, IT, NCI, W2], BF16, name="uh")
                ft = [fpool.tile([P, USZ], BF16, name=f"f{k}")
                      for k in range(5)]
                fA, fB, fC, fD, fE = ft

                def pl(p, s):     # plane p rows [s : s+14], flat [P, USZ]
                    return xt[:, p, s:s + IT].rearrange(
                        "p i c w -> p (i c w)")

                def u(l):
                    return uh[:, l].rearrange("p i c w -> p (i c w)")

                Y0, Y1, Y2, Y3 = pl(0, 0), pl(1, 0), pl(2, 0), pl(3, 0)
                Z0, Z1 = pl(0, 1), pl(1, 1)
                tt = nc.vector.tensor_tensor
                ts = nc.vector.tensor_scalar_mul
                amul = (lambda o, i, m: ts(o, i, m)) if startup else \
                    (lambda o, i, m: nc.scalar.mul(o, i, m))
                il = interleave if interleave else (lambda l: None)
                tt(fA[:], Y1, Y2, op=ALU.add)           # q1
                tt(fB[:], Y1, Y2, op=ALU.subtract)      # r1
                tt(fC[:], Y0, Y2, op=ALU.subtract)      # w1
                tt(fD[:], Z0, Y2, op=ALU.subtract)      # tc
                il(0)
                ts(fE[:], fC[:], 4.0)                   # w14
                tt(u(0), fE[:], fD[:], op=ALU.add)      # u0 = 4*w1 + tc
                il(1)
                ts(fC[:], fA[:], 4.0)                   # q14 (w1 dead)
                tt(fA[:], Y3, Z0, op=ALU.add)           # q2  (q1 dead)
                tt(u(1), fA[:], fC[:], op=ALU.subtract)  # u1 = q2 - 4*q1
                il(2)
                ts(fC[:], fB[:], 4.0)                   # r14 (q14 dead)
                tt(fB[:], Y3, Z0, op=ALU.subtract)      # r2  (r1 dead)
                tt(u(2), fC[:], fB[:], op=ALU.subtract)  # u2 = 4*r1 - r2
                il(3)
                tt(fA[:], Y1, Y3, op=ALU.subtract)      # tp  (q2 dead)
                amul(fB[:], fA[:], 2.0)                 # tp2 (r2 dead)
                tt(u(3), fD[:], fB[:], op=ALU.subtract)  # u3 = tc - 2*tp
                il(4)
                tt(u(4), fD[:], fB[:], op=ALU.add)      # u4 = tc + 2*tp
                amul(fC[:], fA[:], 4.0)                 # tp4
                tt(fD[:], Y3, Z1, op=ALU.subtract)      # q5  (tc dead)
                tt(u(5), fC[:], fD[:], op=ALU.subtract)  # u5 = 4*tp - q5
                return uh

            uhs = fwd(xt0, startup=True, interleave=combos)

            # ---- main loop ----------------------------------------------
            for n in range(bp):
                last = n + 1 == bp
                if not last:
                    nxt_xt = x_load(n + 1)
                m = mpool.tile([P, NL, NCO, 2, NFREE], BF16, name="m")
                for co in range(NCO):
                    for hb in range(2):
                        i0 = hb * IB
                        for lp in range(NL // 2):
                            ps = pspool.tile([P, 2, 512], F32, name="ps")
                            for j in range(2):
                                l = 2 * lp + j
                                for c1 in range(NCI):
                                    for kw in range(KW):
                                        nc.tensor.matmul(
                                            ps[:, j, :NFREE],
                                            lhsT(c1, l, kw, co),
                                            uhs[:, l, i0:i0 + IB, c1,
                                                kw:kw + W],
                                            start=(c1 == 0 and kw == 0),
                                            stop=(c1 == NCI - 1
                                                  and kw == KW - 1),
                                        )
                            # fused evict+scale of the l-pair on ACT
                            nc.scalar.mul(
                                m[:, 2 * lp:2 * lp + 2, co, hb, :],
                                ps[:, :, :NFREE],
                                a_all[:, co:co + 1])
                    if not last and co == 0:
                        nxt_uh = fwd(nxt_xt)
                # inverse transform y = A^T m (co-merged; per-co on last
                # image to shorten the tail), Pool computes e/o.
                cosl = [slice(None)] if not last else \
                    [slice(co, co + 1) for co in range(NCO)]
                y = ypool.tile([P, M, NCO, IT, W], BF16, name="y")
                for cs in cosl:
                    ncs = NCO if cs == slice(None) else 1
                    sz = ncs * 2 * NFREE
                    mv = [m[:, l, cs].rearrange("p c h w -> p (c h w)")
                          for l in range(NL)]
                    tiles = [ipool.tile([P, NCO, 2, NFREE], BF16, name=nm)
                             for nm in ("e", "o", "f", "g", "t")]
                    ev, ov, fv, gv, tv = [
                        q[:].rearrange("p c h w -> p (c h w)")[:, :sz]
                        for q in tiles]
                    itt = nc.vector.tensor_tensor if last else \
                        nc.gpsimd.tensor_tensor
                    imul = (lambda o, i, mm: nc.vector.tensor_scalar_mul(
                        o, i, mm)) if last else \
                        (lambda o, i, mm: nc.scalar.mul(o, i, mm))
                    dtt = nc.vector.tensor_tensor
                    itt(ev, mv[1], mv[2], op=ALU.add)
                    itt(ov, mv[1], mv[2], op=ALU.subtract)
                    dtt(fv, mv[3], mv[4], op=ALU.add)
                    dtt(gv, mv[3], mv[4], op=ALU.subtract)

                    def yv(r):
                        return y[:, r, cs].rearrange("p c h w -> p (c h w)")

                    dtt(tv, mv[0], ev, op=ALU.add)
                    dtt(yv(0), tv, fv, op=ALU.add)
                    imul(tv, fv, 4.0)                   # f4 (t dead)
                    dtt(yv(2), tv, ev, op=ALU.add)      # y2 = 4*f + e
                    imul(ev, gv, 2.0)                   # g2 (e dead)
                    dtt(yv(1), ev, ov, op=ALU.add)      # y1 = 2*g + o
                    dtt(tv, mv[5], ov, op=ALU.add)      # t2 (f4 dead)
                    imul(fv, gv, 8.0)                   # g8 (f dead)
                    dtt(yv(3), fv, tv, op=ALU.add)      # y3 = 8*g + t2
                for co in range(NCO):
                    nc.scalar.dma_start(
                        out[n, co * P:(co + 1) * P, :, :, :], y[:, :, co])
                if not last:
                    uhs = nxt_uh

    nc.compile()
    return nc


_NC_CACHE: dict[int, object] = {}


def _get_nc(bp: int = BP):
    if bp not in _NC_CACHE:
        _NC_CACHE[bp] = build(bp)
    return _NC_CACHE[bp]


def make_in_maps(x: np.ndarray, weight: np.ndarray, n_cores: int = N_CORES,
                 bp: int = BP):
    x = np.ascontiguousarray(x, dtype=np.float32)
    weight = np.ascontiguousarray(weight, dtype=np.float32)
    xb = x.astype(ml_dtypes.bfloat16)
    # phase-major padded layout: x4[n, pc, p, i, c1, w2]
    xpad = np.zeros((B, C, M * NPI, W2), dtype=ml_dtypes.bfloat16)
    xpad[:, :, 1:H + 1, 1:W + 1] = xb
    x4 = np.ascontiguousarray(
        xpad.reshape(B, NCI, P, NPI, M, W2).transpose(0, 2, 4, 3, 1, 5))
    wp = np.ascontiguousarray(
        weight.reshape(O, C, KH * KW).transpose(2, 1, 0)
    ).astype(ml_dtypes.bfloat16)  # [t, i, o]
    wb = weight.reshape(O, KIN).astype(ml_dtypes.bfloat16)
    return [
        {"x4": x4[i * bp:(i + 1) * bp], "wp": wp, "wb": wb}
        for i in range(n_cores)
    ]


def kernel(x: np.ndarray, weight: np.ndarray) -> np.ndarray:
    nc = _get_nc(BP)
    in_maps = make_in_maps(x, weight)
    res = run_bass_kernel_spmd(nc, in_maps, core_ids=list(range(N_CORES)))
    out = np.empty((B, O, H, W), dtype=np.float32)
    for i in range(N_CORES):
        o4 = res.results[i]["out"].astype(np.float32)      # [bp,O,4,14,56]
        out[i * BP:(i + 1) * BP] = (
            o4.reshape(BP, O, M, IT, W).transpose(0, 1, 3, 2, 4)
            .reshape(BP, O, H, W))
    return out
